# revision 2
# baseline (speedup 1.0000x reference)
"""Trainium2 Bass kernel for the dual-pass (inter/intra) MultiHeadAttention module.

Contract: kernel(**inputs) takes FULL unsharded numpy inputs (keys as in
setup_inputs()) and returns the FULL [32, 512, 512] float32 output.

Sharding: data-parallel over batch. 8 cores x 4 batch elements each; all
weights replicated; no collectives. Host pre-transposes/fuses weights and
converts matmul operands to bf16 (fp32 PSUM accumulation keeps the result
well inside the 2e-2 gate), gathers per-core outputs.

Host-side weight fusion removes two of the seven 512^3 GEMMs per batch:
  Win  = Wsi @ Wp          (a2T  = Win @ x^T directly)
  Wout = Wpo @ Woi         (out_inter_nat = Wout @ concat^T + x)
and the PE-transpose stage is replaced by a second chain off the same
concat tiles (oi_t = (Wout @ cT)^T + x^T, with x^T DMA'd pre-transposed).

Per-core dataflow (per batch element, activations feature-major [feat, tok]):
  a2T = Win @ x                       (chain512)
  cT  = MHA_inter(a2T)                per-head-pair block-diag QKV
  oi_n = Wout @ cT + x                (natural [s,d], for residual/final)
  oi_t = (Wout @ cT)^T + x^T          (chain with cT as lhsT)
  xiT = Wsa @ oi_t                    (chain512)
  c2T = MHA_intra(xiT, mask)
  y   = (1-a)*Woa-chain(c2T) + a*oi_n ((1-a) folded into Woa on host)
"""

import os
import sys
from contextlib import ExitStack

import numpy as np

sys.path.insert(0, "/opt/trn_rl_repo")

from concourse import bass, bacc, mybir, tile  # noqa: E402
from concourse.bass_utils import run_bass_kernel_spmd  # noqa: E402

B, S, D = 32, 512, 512
H, HD = 8, 64
NCORES = 8
BPC = B // NCORES  # batches per core
P = 128  # partitions
NT = D // P  # 4 tiles per 512 axis

F32 = mybir.dt.float32
BF16 = mybir.dt.bfloat16

# test-only knob: repeat the per-batch pipeline N times (for differential timing)
REPEAT = int(os.environ.get("BASS_REPEAT", "1"))


def build_bass(a_val: float, with_mask: bool):
    """Build the single-core SPMD program. a_val = sigmoid(alpha)."""
    nc = bacc.Bacc(
        "TRN2",
        target_bir_lowering=False,
        debug=False,
        enable_asserts=False,
        num_devices=NCORES,
    )

    xb_d = nc.dram_tensor("xb", [BPC, S, D], BF16, kind="ExternalInput")
    xtb_d = nc.dram_tensor("xtb", [BPC, D, S], BF16, kind="ExternalInput")
    w_names = [
        ("WinT", [D, D]),
        ("WoutT", [D, D]),
        ("WsaT", [D, D]),
        ("WoaT", [D, D]),
        ("WqPi", [H // 2, P, P]),
        ("WkPi", [H // 2, P, P]),
        ("WvPi", [H // 2, P, P]),
        ("WqPa", [H // 2, P, P]),
        ("WkPa", [H // 2, P, P]),
        ("WvPa", [H // 2, P, P]),
    ]
    wd = {n: nc.dram_tensor(n, shp, BF16, kind="ExternalInput") for n, shp in w_names}
    if with_mask:
        wd["maskT"] = nc.dram_tensor("maskT", [S, S], F32, kind="ExternalInput")
    y_d = nc.dram_tensor("y", [BPC, S, D], F32, kind="ExternalOutput")

    EXP = mybir.ActivationFunctionType.Exp
    MULT = mybir.AluOpType.mult
    ADD = mybir.AluOpType.add

    with tile.TileContext(nc) as tc, ExitStack() as ctx:
        ctx.enter_context(
            nc.allow_low_precision(reason="bf16 matmul operands, fp32 PSUM accum")
        )
        wpool = ctx.enter_context(tc.tile_pool(name="weights", bufs=1))
        apool = ctx.enter_context(tc.tile_pool(name="acts", bufs=2))
        dpool = ctx.enter_context(tc.tile_pool(name="scratch", bufs=2))
        pspool = ctx.enter_context(tc.tile_pool(name="psum", bufs=8, space="PSUM"))

        def ps(shape, tag, bufs):
            return pspool.tile(shape, F32, tag=tag, name=tag, bufs=bufs)

        def mm(out, lhsT, rhs, start=None, stop=None):
            nc.tensor.matmul(out, lhsT, rhs, start=start, stop=stop)

        # ---- persistent weights in SBUF (DMA'd directly as bf16) ----
        def load_big(name, dt=BF16):  # [512,512] -> 4 x [128,512]
            tiles = []
            for k in range(NT):
                t = wpool.tile([P, 512], dt, tag=f"{name}{k}", name=f"{name}{k}")
                nc.sync.dma_start(t[:], wd[name][k * P : (k + 1) * P, :])
                tiles.append(t)
            return tiles

        def load_pairs(name):
            """[4,128,128] block-diagonal pair weights -> 4 tiles [128,128]."""
            tiles = []
            for g in range(H // 2):
                t = wpool.tile([P, P], BF16, tag=f"{name}{g}", name=f"{name}{g}")
                nc.sync.dma_start(t[:], wd[name][g])
                tiles.append(t)
            return tiles

        winT = load_big("WinT")
        wqPi = load_pairs("WqPi")
        wkPi = load_pairs("WkPi")
        wvPi = load_pairs("WvPi")
        woutT = load_big("WoutT")
        wsaT = load_big("WsaT")
        wqPa = load_pairs("WqPa")
        wkPa = load_pairs("WkPa")
        wvPa = load_pairs("WvPa")
        woaT = load_big("WoaT")

        ones_f32 = wpool.tile([P, 1], F32, tag="ones_f32", name="ones_f32")
        nc.vector.memset(ones_f32[:], 1.0)

        maskT = None
        if with_mask:
            maskT = load_big("maskT", dt=F32)

        # ---- helpers ----
        def chain512(lhsT_tiles, rhs_tiles, out_tag, out_dt=BF16, copy_eng="vector"):
            """out[m-chunk] = sum_k lhsT_tiles[k][:, m]^T @ rhs_tiles[k].
            Returns 4 x [128, 512] SBUF tiles."""
            outs = []
            for m in range(NT):
                acc = ps([P, 512], tag="acc", bufs=2)
                for k in range(NT):
                    mm(
                        acc[:],
                        lhsT_tiles[k][:, m * P : (m + 1) * P],
                        rhs_tiles[k][:],
                        start=(k == 0),
                        stop=(k == NT - 1),
                    )
                o = apool.tile([P, 512], out_dt, tag=f"{out_tag}{m}", name=f"{out_tag}{m}")
                if copy_eng == "vector":
                    nc.vector.tensor_copy(o[:], acc[:])
                else:
                    nc.scalar.copy(o[:], acc[:])
                outs.append(o)
            return outs

        def mha(inT, wqP, wkP, wvP, concat_tag, use_mask):
            """inT: 4 x [128,512] transposed activations [(h,e), n].
            Head-pair packing: pair g = heads (2g, 2g+1) lives in inT[g];
            block-diagonal pair weights compute both heads per matmul.
            Returns concatT: 4 x [128,512] [(h,e), n] bf16."""
            concatT = [
                apool.tile(
                    [P, 512], BF16, tag=f"{concat_tag}{g}", name=f"{concat_tag}{g}"
                )
                for g in range(NT)
            ]
            for g in range(H // 2):
                src = inT[g]  # [128, 512] = both heads of the pair
                pq = ps([P, 512], tag="qk", bufs=2)
                mm(pq[:], wqP[g][:], src[:])
                qp = dpool.tile([P, 512], BF16, tag="qp", name="qp")
                nc.vector.tensor_copy(qp[:], pq[:])
                pk = ps([P, 512], tag="qk", bufs=2)
                mm(pk[:], wkP[g][:], src[:])
                kp = dpool.tile([P, 512], BF16, tag="kp", name="kp")
                nc.vector.tensor_copy(kp[:], pk[:])
                # v for both heads: pv4[:, mc*128+c] c<64 head A, c>=64 head B
                pv4 = ps([P, 512], tag="sv", bufs=2)
                for mc in range(NT):
                    mm(
                        pv4[:, mc * P : (mc + 1) * P],
                        src[:, mc * P : (mc + 1) * P],
                        wvP[g][:],
                    )
                # v4p layout per chunk: [vA(64) | 1 | vB(64) | 1] = 130 cols,
                # so lhsT slices [:, mc, 0:65] / [:, mc, 65:130] are contiguous.
                v4p = dpool.tile([P, NT, 2, HD + 1], BF16, tag="v4p", name="v4p")
                nc.vector.tensor_copy(
                    v4p[:, :, :, 0:HD],
                    pv4[:].rearrange("p (a h c) -> p a h c", a=NT, h=2),
                )
                nc.gpsimd.tensor_copy(
                    v4p[:, :, :, HD : HD + 1],
                    ones_f32[:, 0:1].broadcast_to([P, NT, 2, 1]),
                )
                v4v = v4p[:].rearrange("p a h c -> p a (h c)")
                for hh in range(2):
                    h0 = hh * HD
                    qT = qp[h0 : h0 + HD, :]
                    kT = kp[h0 : h0 + HD, :]
                    # S^T chunks -> P^T = exp(S^T / 8)
                    pts = []
                    for mc in range(NT):
                        s_ps = ps([P, 512], tag="sv", bufs=2)
                        mm(s_ps[:], kT[:, mc * P : (mc + 1) * P], qT[:])
                        pt = dpool.tile([P, 512], BF16, tag=f"pt{mc}", name=f"pt{mc}")
                        if use_mask:
                            tmp = dpool.tile(
                                [P, 512], F32, tag=f"mtmp{mc}", name=f"mtmp{mc}"
                            )
                            nc.vector.scalar_tensor_tensor(
                                tmp[:], s_ps[:], 0.125, maskT[mc][:], MULT, ADD
                            )
                            nc.scalar.activation(pt[:], tmp[:], EXP)
                        else:
                            nc.scalar.activation(pt[:], s_ps[:], EXP, scale=0.125)
                        pts.append(pt)
                    # o_aug^T [65, n], accumulate over m-chunks
                    po = ps([HD + 1, 512], tag="o", bufs=2)
                    for mc in range(NT):
                        mm(
                            po[:],
                            v4v[:, mc, h0 + hh : h0 + hh + HD + 1],
                            pts[mc][:],
                            start=(mc == 0),
                            stop=(mc == NT - 1),
                        )
                    # normalize rows 0..63 by row 64: reciprocal of the rowsum
                    # row, partition-broadcast on Pool, one DVE multiply.
                    rec = dpool.tile([1, 512], F32, tag="rec", name="rec")
                    nc.vector.reciprocal(rec[:], po[HD : HD + 1, :])
                    bc = dpool.tile([HD, 512], F32, tag="bc", name="bc")
                    nc.gpsimd.partition_broadcast(bc[:], rec[:])
                    nc.vector.tensor_mul(
                        concatT[g][h0 : h0 + HD, :],
                        po[0:HD, :],
                        bc[:],
                    )
            return concatT

        # ---- per-batch pipeline ----
        def load_x(b):
            xs, xts = [], []
            for m in range(NT):
                t = apool.tile([P, 512], BF16, tag=f"xb{m}", name=f"xb{m}")
                nc.sync.dma_start(t[:], xb_d[b, m * P : (m + 1) * P, :])
                xs.append(t)
            for m in range(NT):
                t = apool.tile([P, 512], BF16, tag=f"xt{m}", name=f"xt{m}")
                nc.sync.dma_start(t[:], xtb_d[b, m * P : (m + 1) * P, :])
                xts.append(t)
            return xs, xts

        seq = [bb % BPC for bb in range(BPC * REPEAT)]
        xcache = {0: load_x(seq[0])}
        for bi, b in enumerate(seq):
            x_sb, xt_sb = xcache.pop(bi)
            if bi + 1 < len(seq):
                xcache[bi + 1] = load_x(seq[bi + 1])

            a2T = chain512(winT, x_sb, "a2T", copy_eng="scalar")  # [e, d]
            cT = mha(a2T, wqPi, wkPi, wvPi, "cT", use_mask=False)

            # out_inter natural [s, d] = Wout @ cT + x
            oi_n = []
            for m in range(NT):
                acc = ps([P, 512], tag="acc", bufs=2)
                for k in range(NT):
                    mm(
                        acc[:],
                        woutT[k][:, m * P : (m + 1) * P],
                        cT[k][:],
                        start=(k == 0),
                        stop=(k == NT - 1),
                    )
                o = apool.tile([P, 512], F32, tag=f"oin{m}", name=f"oin{m}")
                nc.vector.tensor_add(o[:], acc[:], x_sb[m][:])
                oi_n.append(o)

            # out_inter transposed [d, s] = (Wout @ cT)^T + x^T
            oi_t = []
            for m in range(NT):
                acc = ps([P, 512], tag="acc", bufs=2)
                for k in range(NT):
                    mm(
                        acc[:],
                        cT[k][:, m * P : (m + 1) * P],
                        woutT[k][:],
                        start=(k == 0),
                        stop=(k == NT - 1),
                    )
                o = apool.tile([P, 512], BF16, tag=f"oit{m}", name=f"oit{m}")
                nc.vector.tensor_add(o[:], acc[:], xt_sb[m][:])
                oi_t.append(o)

            xiT = chain512(wsaT, oi_t, "xiT", copy_eng="scalar")  # [e, s]
            c2T = mha(xiT, wqPa, wkPa, wvPa, "c2T", use_mask=with_mask)

            # out natural [s, f] accumulate over c; Woa pre-scaled by (1-a)
            for m in range(NT):
                acc = ps([P, 512], tag="acc", bufs=2)
                for k in range(NT):
                    mm(
                        acc[:],
                        c2T[k][:, m * P : (m + 1) * P],
                        woaT[k][:],
                        start=(k == 0),
                        stop=(k == NT - 1),
                    )
                fin = apool.tile([P, 512], F32, tag=f"fin{m}", name=f"fin{m}")
                # fin = out*(1-a) [already folded] + a*out_inter
                nc.vector.scalar_tensor_tensor(
                    fin[:], oi_n[m][:], float(a_val), acc[:], MULT, ADD
                )
                nc.sync.dma_start(y_d[b, m * P : (m + 1) * P, :], fin[:])

    nc.compile()
    return nc


def _prep_inputs(inputs):
    """Host-side: sigmoid(alpha), weight fusion/transposes, per-core maps."""
    import ml_dtypes

    f32 = np.float32
    bf16 = ml_dtypes.bfloat16

    def t2(w):  # [out,in] -> [in,out]
        return np.ascontiguousarray(np.asarray(w, f32).T)

    def pairblk(w):
        """[8,64,64] per-head W -> [4,128,128] block-diag pair lhsT:
        blkdiag(W[2g].T, W[2g+1].T)."""
        wt = np.transpose(np.asarray(w, f32), (0, 2, 1))
        out = np.zeros((H // 2, P, P), f32)
        for g in range(H // 2):
            out[g, :HD, :HD] = wt[2 * g]
            out[g, HD:, HD:] = wt[2 * g + 1]
        return out.astype(bf16)

    a_val = float(1.0 / (1.0 + np.exp(-np.float32(inputs["alpha"]))))
    mask = np.asarray(inputs["mask"], f32)
    with_mask = bool(np.any(mask))

    wp = np.asarray(inputs["W_proj_in"], f32)
    wsi = np.asarray(inputs["W_split_inter"], f32)
    woi = np.asarray(inputs["W_out_inter"], f32)
    wpo = np.asarray(inputs["W_proj_out"], f32)
    win = wsi @ wp  # a2 = Win @ x^T
    wout = wpo @ woi  # out_inter = Wout @ concat^T

    common = {
        "WinT": t2(win).astype(bf16),
        "WoutT": t2(wout).astype(bf16),
        "WsaT": t2(inputs["W_split_intra"]).astype(bf16),
        "WoaT": np.ascontiguousarray(
            (np.asarray(inputs["W_out_intra"], f32) * f32(1.0 - a_val)).T
        ).astype(bf16),
        "WqPi": pairblk(inputs["Wq_inter"]),
        "WkPi": pairblk(inputs["Wk_inter"]),
        "WvPi": pairblk(inputs["Wv_inter"]),
        "WqPa": pairblk(inputs["Wq_intra"]),
        "WkPa": pairblk(inputs["Wk_intra"]),
        "WvPa": pairblk(inputs["Wv_intra"]),
    }
    if with_mask:
        common["maskT"] = np.ascontiguousarray(mask.T)

    x = np.asarray(inputs["x"], f32)
    xb = x.astype(bf16)
    xtb = np.ascontiguousarray(np.swapaxes(x, 1, 2)).astype(bf16)
    in_maps = []
    for c in range(NCORES):
        m = dict(common)
        m["xb"] = np.ascontiguousarray(xb[c * BPC : (c + 1) * BPC])
        m["xtb"] = np.ascontiguousarray(xtb[c * BPC : (c + 1) * BPC])
        in_maps.append(m)
    return a_val, with_mask, in_maps


def _run(inputs, trace=False):
    a_val, with_mask, in_maps = _prep_inputs(inputs)
    nc = build_bass(a_val, with_mask)
    res = run_bass_kernel_spmd(
        nc,
        in_maps,
        core_ids=list(range(NCORES)),
        trace=trace,
    )
    out = np.concatenate([res.results[c]["y"] for c in range(NCORES)], axis=0)
    return out.astype(np.float32), res


def kernel(**inputs):
    out, _ = _run(inputs, trace=False)
    return out


# revision 18
# speedup vs baseline: 1.3790x; 1.3790x over previous
"""Trainium2 Bass kernel for the dual-pass (inter/intra) MultiHeadAttention module.

Contract: kernel(**inputs) takes FULL unsharded numpy inputs (keys as in
setup_inputs()) and returns the FULL [32, 512, 512] float32 output.

Sharding: data-parallel over batch. 8 cores x 4 batch elements each; all
weights replicated; no collectives. Host pre-transposes/fuses weights and
converts matmul operands to bf16 (fp32 PSUM accumulation keeps the result
well inside the 2e-2 gate), gathers per-core outputs.

Host-side weight fusion removes two of the seven 512^3 GEMMs per batch:
  Win  = Wsi @ Wp          (a2T  = Win @ x^T directly)
  Wout = Wpo @ Woi         (out_inter_nat = Wout @ concat^T + x)
and the PE-transpose stage is replaced by a second chain off the same
concat tiles (oi_t = (Wout @ cT)^T + x^T, with x^T DMA'd pre-transposed).

Math per batch element (activations feature-major [feat, tok]):
  A: a2T = Win @ x                    (chain512)
  B: cT  = MHA_inter(a2T)             per-head-pair block-diag QKV
  C: oi_n = Wout @ cT + x  [s,d];  oi_t = (Wout @ cT)^T + x^T  [d,s]
  D: xiT = Wsa @ oi_t
  E: c2T = MHA_intra(xiT, mask)
  F: y   = Woa-chain(c2T)*(1-a) + a*oi_n   ((1-a) folded into Woa on host)

Scheduling: engines execute their instruction stream in emission order, so
the emitter software-pipelines two batches: slot i emits
  C(i), av-drain, F(i-1), D(i), A(i+1), pairs[B.g(i+1) / E.g(i) alternating]
with attention av-matmuls lagging one pair behind their S/exp stage so the
Act engine's exp latency never blocks PE.
"""

import os
import sys
from contextlib import ExitStack

import numpy as np

sys.path.insert(0, "/opt/trn_rl_repo")

from concourse import bass, bacc, mybir, tile  # noqa: E402
from concourse.bass_utils import run_bass_kernel_spmd  # noqa: E402

B, S, D = 32, 512, 512
H, HD = 8, 64
NCORES = 8
BPC = B // NCORES  # batches per core
P = 128  # partitions
NT = D // P  # 4 tiles per 512 axis

F32 = mybir.dt.float32
BF16 = mybir.dt.bfloat16

# test-only knob: repeat the per-batch pipeline N times (for differential timing)
REPEAT = int(os.environ.get("BASS_REPEAT", "1"))
# PSUM layout: "big" shares one 4-deep ring for chain accs + q/k + v; "sep"
# gives chains and q/k their own 2-deep rings.
PS_MODE = os.environ.get("BASS_PS", "big")


def build_bass(a_val: float, with_mask: bool):
    """Build the single-core SPMD program. a_val = sigmoid(alpha)."""
    nc = bacc.Bacc(
        "TRN2",
        target_bir_lowering=False,
        debug=False,
        enable_asserts=False,
        num_devices=NCORES,
    )

    xb_d = nc.dram_tensor("xb", [BPC, S, D], BF16, kind="ExternalInput")
    xtb_d = nc.dram_tensor("xtb", [BPC, D, S], BF16, kind="ExternalInput")
    w_names = [
        ("WinT", [D, D]),
        ("WoutT", [D, D]),
        ("WsaT", [D, D]),
        ("WoaT", [D, D]),
        ("WqPi", [H // 2, P, P]),
        ("WkPi", [H // 2, P, P]),
        ("WvPi", [H // 2, P, P]),
        ("WqPa", [H // 2, P, P]),
        ("WkPa", [H // 2, P, P]),
        ("WvPa", [H // 2, P, P]),
        ("ident", [P, P]),
    ]
    wd = {n: nc.dram_tensor(n, shp, BF16, kind="ExternalInput") for n, shp in w_names}
    if with_mask:
        wd["maskT"] = nc.dram_tensor("maskT", [S, S], F32, kind="ExternalInput")
    y_d = nc.dram_tensor("y", [BPC, S, D], F32, kind="ExternalOutput")

    EXP = mybir.ActivationFunctionType.Exp
    MULT = mybir.AluOpType.mult
    ADD = mybir.AluOpType.add

    with tile.TileContext(nc) as tc, ExitStack() as ctx:
        ctx.enter_context(
            nc.allow_low_precision(reason="bf16 matmul operands, fp32 PSUM accum")
        )
        wpool = ctx.enter_context(tc.tile_pool(name="weights", bufs=1))
        apool = ctx.enter_context(tc.tile_pool(name="acts", bufs=3))
        dpool = ctx.enter_context(tc.tile_pool(name="scratch", bufs=2))
        pspool = ctx.enter_context(tc.tile_pool(name="psum", bufs=8, space="PSUM"))

        PS = {
            "big": {"acc": ("big", 4), "qk": ("big", 4), "pv": ("big", 4),
                    "s": ("sv", 2), "o": ("o", 1), "tp": ("tp", 1)},
            "sep": {"acc": ("acc", 2), "qk": ("qk", 2), "pv": ("sv", 2),
                    "s": ("sv", 2), "o": ("o", 2), "tp": ("tp", 2)},
        }[PS_MODE]

        def ps(shape, role, dt=F32):
            tag, bufs = PS[role]
            return pspool.tile(shape, dt, tag=tag, name=tag, bufs=bufs)

        def mm(out, lhsT, rhs, start=None, stop=None):
            nc.tensor.matmul(out, lhsT, rhs, start=start, stop=stop)

        # ---- x loads first (batch 0 feeds the first chain), then weights ----
        def load_x(b):
            xs, xts = [], []
            for m in range(NT):
                t = apool.tile([P, 512], BF16, tag=f"xb{m}", name=f"xb{m}")
                nc.sync.dma_start(t[:], xb_d[b, m * P : (m + 1) * P, :])
                xs.append(t)
            for m in range(NT):
                t = apool.tile([P, 512], BF16, tag=f"xt{m}", name=f"xt{m}")
                nc.sync.dma_start(t[:], xtb_d[b, m * P : (m + 1) * P, :])
                xts.append(t)
            return xs, xts

        def load_big(name, dt=BF16):  # [512,512] -> 4 x [128,512]
            tiles = []
            for k in range(NT):
                t = wpool.tile([P, 512], dt, tag=f"{name}{k}", name=f"{name}{k}")
                nc.sync.dma_start(t[:], wd[name][k * P : (k + 1) * P, :])
                tiles.append(t)
            return tiles

        def load_pairs(name):
            tiles = []
            for g in range(H // 2):
                t = wpool.tile([P, P], BF16, tag=f"{name}{g}", name=f"{name}{g}")
                nc.sync.dma_start(t[:], wd[name][g])
                tiles.append(t)
            return tiles

        seq = [bb % BPC for bb in range(BPC * REPEAT)]
        NSEQ = len(seq)
        xb0 = []
        for m in range(NT):
            t = apool.tile([P, 512], BF16, tag=f"xb{m}", name=f"xb{m}")
            nc.sync.dma_start(t[:], xb_d[seq[0], m * P : (m + 1) * P, :])
            xb0.append(t)
        winT = load_big("WinT")
        xt0 = []
        for m in range(NT):
            t = apool.tile([P, 512], BF16, tag=f"xt{m}", name=f"xt{m}")
            nc.sync.dma_start(t[:], xtb_d[seq[0], m * P : (m + 1) * P, :])
            xt0.append(t)
        xtiles = {0: (xb0, xt0)}

        wq = {"i": load_pairs("WqPi"), "a": load_pairs("WqPa")}
        wk = {"i": load_pairs("WkPi"), "a": load_pairs("WkPa")}
        wv = {"i": load_pairs("WvPi"), "a": load_pairs("WvPa")}
        woutT = load_big("WoutT")
        wsaT = load_big("WsaT")
        woaT = load_big("WoaT")

        ident = wpool.tile([P, P], BF16, tag="ident", name="ident")
        nc.sync.dma_start(ident[:], wd["ident"][:])
        ones_f32 = wpool.tile([P, 1], F32, tag="ones_f32", name="ones_f32")
        nc.vector.memset(ones_f32[:], 1.0)

        maskT = None
        if with_mask:
            maskT = load_big("maskT", dt=F32)

        if 1 < NSEQ:
            xtiles[1] = load_x(seq[1])

        # ---- per-batch state ----
        st = [dict() for _ in range(NSEQ)]

        def emit_A_chunk(i, m):
            """a2T[m] = (Win @ x)[m-chunk] (bf16, drained on Act)."""
            x_sb, _ = xtiles[i]
            acc = ps([P, 512], "acc")
            for k in range(NT):
                mm(acc[:], winT[k][:, m * P : (m + 1) * P], x_sb[k][:],
                   start=(k == 0), stop=(k == NT - 1))
            o = apool.tile([P, 512], BF16, tag=f"a2T{m}", name=f"a2T{m}")
            (nc.scalar.copy if m % 2 == 0 else nc.vector.tensor_copy)(o[:], acc[:])
            st[i].setdefault("a2T", [None] * NT)[m] = o
            if m == NT - 1:
                st[i]["cT"] = [None] * NT

        def emit_A(i):
            for m in range(NT):
                emit_A_chunk(i, m)

        def emit_prelude(i, mh, g):
            """q/k/v projections for pair g of MHA phase mh on batch-index i."""
            s = st[i]
            src = (s["a2T"] if mh == "i" else s["xiT"])[g]
            pq = ps([P, 512], "qk")
            mm(pq[:], wq[mh][g][:], src[:])
            qp = dpool.tile([P, 512], BF16, tag="qp", name="qp")
            nc.vector.tensor_copy(qp[:], pq[:])
            pk = ps([P, 512], "qk")
            mm(pk[:], wk[mh][g][:], src[:])
            kp = dpool.tile([P, 512], BF16, tag="kp", name="kp")
            nc.vector.tensor_copy(kp[:], pk[:])
            pv4 = ps([P, 512], "pv")
            for mc in range(NT):
                mm(pv4[:, mc * P : (mc + 1) * P], src[:, mc * P : (mc + 1) * P],
                   wv[mh][g][:])
            # v4p chunk layout [vA(64) | 1 | vB(64) | 1] -> contiguous 65-wide
            # lhsT slices per head.
            v4p = dpool.tile([P, NT, 2, HD + 1], BF16, tag="v4p", name="v4p")
            nc.vector.tensor_copy(
                v4p[:, :, :, 0:HD],
                pv4[:].rearrange("p (a h c) -> p a h c", a=NT, h=2),
            )
            nc.gpsimd.tensor_copy(
                v4p[:, :, :, HD : HD + 1],
                ones_f32[:, 0:1].broadcast_to([P, NT, 2, 1]),
            )
            # concat output tile for this pair
            ctag = "cT" if mh == "i" else "c2T"
            cten = apool.tile([P, 512], BF16, tag=f"{ctag}{g}", name=f"{ctag}{g}")
            s[ctag][g] = cten
            s[f"pair{mh}{g}"] = {"qp": qp, "kp": kp, "v4p": v4p, "pts": {}}

        def emit_S(i, mh, g, hh, use_mask):
            """S^T chunks + exp for head hh of pair g."""
            pr = st[i][f"pair{mh}{g}"]
            h0 = hh * HD
            qT = pr["qp"][h0 : h0 + HD, :]
            kT = pr["kp"][h0 : h0 + HD, :]
            pts = []
            for mc in range(NT):
                s_ps = ps([P, 512], "s")
                mm(s_ps[:], kT[:, mc * P : (mc + 1) * P], qT[:])
                pt = dpool.tile([P, 512], BF16, tag=f"pt{mc}", name=f"pt{mc}",
                                bufs=3)
                if use_mask:
                    tmp = dpool.tile([P, 512], F32, tag=f"mtmp{mc}",
                                     name=f"mtmp{mc}", bufs=3)
                    nc.vector.scalar_tensor_tensor(
                        tmp[:], s_ps[:], 0.125, maskT[mc][:], MULT, ADD
                    )
                    nc.scalar.activation(pt[:], tmp[:], EXP)
                else:
                    nc.scalar.activation(pt[:], s_ps[:], EXP, scale=0.125)
                pts.append(pt)
            pr["pts"][hh] = pts

        def emit_av(i, mh, g, hh):
            """attn@v in natural [n, e+1] orientation (streams 65 cols/pass
            instead of 512), per-partition softmax normalize, PE transpose
            back to the concat layout."""
            s = st[i]
            pr = s[f"pair{mh}{g}"]
            v4v = pr["v4p"][:].rearrange("p a h c -> p a (h c)")
            h0 = hh * HD
            po = ps([P, NT, HD + 1], "o")
            for nchunk in range(NT):
                for mc in range(NT):
                    mm(po[:, nchunk, :],
                       pr["pts"][hh][mc][:, nchunk * P : (nchunk + 1) * P],
                       v4v[:, mc, h0 + hh : h0 + hh + HD + 1],
                       start=(mc == 0), stop=(mc == NT - 1))
            rec4 = dpool.tile([P, NT, 1], F32, tag="rec4", name="rec4")
            nc.vector.reciprocal(rec4[:], po[:, :, HD : HD + 1])
            pn = dpool.tile([P, NT, HD], BF16, tag="pn", name="pn")
            nc.vector.scalar_tensor_tensor(
                pn[:], po[:, :, 0:HD], 1.0,
                rec4[:].broadcast_to([P, NT, HD]), MULT, MULT,
            )
            tpo = ps([HD, NT, P], "tp", dt=BF16)
            for nchunk in range(NT):
                nc.tensor.transpose(tpo[:, nchunk, :], pn[:, nchunk, :], ident[:])
            ctag = "cT" if mh == "i" else "c2T"
            nc.vector.tensor_copy(
                s[ctag][g][h0 : h0 + HD, :],
                tpo[:].rearrange("p a q -> p (a q)"),
            )

        def emit_oit_chunk(i, m):
            """oi_t[m] = X^T[m-chunk of d] + x^T[m], via PE transposes of Xbf."""
            s = st[i]
            tp = ps([P, NT, P], "acc", dt=BF16)
            for j in range(NT):
                nc.tensor.transpose(
                    tp[:, j, :], s["Xbf"][j][:, m * P : (m + 1) * P], ident[:]
                )
            o = apool.tile([P, 512], BF16, tag=f"oit{m}", name=f"oit{m}")
            nc.vector.tensor_add(
                o[:], tp[:].rearrange("p a q -> p (a q)"), xtiles[i][1][m][:]
            )
            s.setdefault("oi_t", [None] * NT)[m] = o
            if m == NT - 1:
                del xtiles[i]

        def emit_oin_chunk(i, m):
            """oi_n[m] = (Wout @ cT + x)[m-chunk of s] (f32 for the final mix).
            Also drains X[m] = (Wout @ cT)[m] as bf16 for the PE transposes
            that build oi_t."""
            s = st[i]
            acc = ps([P, 512], "acc")
            for k in range(NT):
                mm(acc[:], woutT[k][:, m * P : (m + 1) * P], s["cT"][k][:],
                   start=(k == 0), stop=(k == NT - 1))
            o = apool.tile([P, 512], F32, tag=f"oin{m}", name=f"oin{m}")
            nc.vector.tensor_add(o[:], acc[:], xtiles[i][0][m][:])
            xb = apool.tile([P, 512], BF16, tag=f"Xbf{m}", name=f"Xbf{m}")
            if m % 2 == 0:
                nc.scalar.copy(xb[:], acc[:])
            else:
                nc.vector.tensor_copy(xb[:], acc[:])
            s.setdefault("oi_n", [None] * NT)[m] = o
            s.setdefault("Xbf", [None] * NT)[m] = xb

        def emit_D_chunk(i, m):
            """xiT[m] = (Wsa @ oi_t)[m-chunk] (bf16, drained on Act)."""
            s = st[i]
            acc = ps([P, 512], "acc")
            for k in range(NT):
                mm(acc[:], wsaT[k][:, m * P : (m + 1) * P], s["oi_t"][k][:],
                   start=(k == 0), stop=(k == NT - 1))
            o = apool.tile([P, 512], BF16, tag=f"xiT{m}", name=f"xiT{m}")
            (nc.scalar.copy if m % 2 == 0 else nc.vector.tensor_copy)(o[:], acc[:])
            s.setdefault("xiT", [None] * NT)[m] = o
            if m == NT - 1:
                s["c2T"] = [None] * NT

        def emit_F_chunk(i, m):
            """y[m] = (Woa-chain(c2T) + a*oi_n)[m-chunk], DMA out."""
            s = st[i]
            acc = ps([P, 512], "acc")
            for k in range(NT):
                mm(acc[:], s["c2T"][k][:, m * P : (m + 1) * P], woaT[k][:],
                   start=(k == 0), stop=(k == NT - 1))
            fin = apool.tile([P, 512], F32, tag=f"fin{m}", name=f"fin{m}")
            nc.vector.scalar_tensor_tensor(
                fin[:], s["oi_n"][m][:], float(a_val), acc[:], MULT, ADD
            )
            nc.sync.dma_start(y_d[seq[i], m * P : (m + 1) * P, :], fin[:])

        CHUNK = {"oit": emit_oit_chunk, "oin": emit_oin_chunk,
                 "D": emit_D_chunk, "F": emit_F_chunk}

        # ---- software-pipelined emission ----
        # pair pump: av matmuls lag one pair behind S/exp
        prev_pair = None  # (i, mh, g)

        def pump_pair(i, mh, g, use_mask):
            nonlocal prev_pair
            emit_prelude(i, mh, g)
            emit_S(i, mh, g, 0, use_mask)
            if prev_pair is not None:
                emit_av(*prev_pair, 0)
            emit_S(i, mh, g, 1, use_mask)
            if prev_pair is not None:
                emit_av(*prev_pair, 1)
            prev_pair = (i, mh, g)

        def drain_pairs():
            nonlocal prev_pair
            if prev_pair is not None:
                emit_av(*prev_pair, 0)
                emit_av(*prev_pair, 1)
                prev_pair = None

        # prologue: batch 0 a2T with k-outer accumulation so the first
        # matmuls start as soon as the first x/WinT DMAs land; then the
        # inter-pass pumps with A(1) chunks interleaved to keep PE fed.
        emit_A(0)
        for g in range(H // 2):
            pump_pair(0, "i", g, False)

        # steady state: per slot, B(i+1)/E(i) pumps with two chain chunks
        # interleaved after each pump so PE always has work while Act
        # drains the exp backlog.
        for i in range(NSEQ):
            nb = i + 1 if i + 1 < NSEQ else None
            # oin before oit (oit transposes read Xbf from oin); all of
            # oin/oit/D before the first E pump (E reads xiT = D).
            cdef = [("oin", i, m) for m in range(NT)]
            cdef += [("oit", i, m) for m in range(NT)]
            fch = [("F", i - 1, m) for m in range(NT)] if i > 0 else []

            if nb is not None:
                for m in range(NT):
                    emit_A_chunk(nb, m)
                if nb + 1 < NSEQ:
                    xtiles[nb + 1] = load_x(seq[nb + 1])
                ci = 0
                for g in range(H // 2):
                    pump_pair(nb, "i", g, False)
                    for _ in range(2):
                        kind, bi, m = cdef[ci]
                        CHUNK[kind](bi, m)
                        ci += 1
                for m in range(NT):
                    emit_D_chunk(i, m)
                ci = 0
                for g in range(H // 2):
                    pump_pair(i, "a", g, with_mask)
                    if ci < len(fch):
                        kind, bi, m = fch[ci]
                        CHUNK[kind](bi, m)
                        ci += 1
                while ci < len(fch):
                    kind, bi, m = fch[ci]
                    CHUNK[kind](bi, m)
                    ci += 1
            else:
                # last slot: no B partner; oin/oit/D first, then the E pumps
                # with the F chunks interleaved
                for kind, bi, m in cdef:
                    CHUNK[kind](bi, m)
                for m in range(NT):
                    emit_D_chunk(i, m)
                ci = 0
                for g in range(H // 2):
                    pump_pair(i, "a", g, with_mask)
                    if ci < len(fch):
                        kind, bi, m = fch[ci]
                        CHUNK[kind](bi, m)
                        ci += 1
                while ci < len(fch):
                    kind, bi, m = fch[ci]
                    CHUNK[kind](bi, m)
                    ci += 1
        drain_pairs()
        for m in range(NT):
            emit_F_chunk(NSEQ - 1, m)

    nc.compile()
    return nc


def _prep_inputs(inputs):
    """Host-side: sigmoid(alpha), weight fusion/transposes, per-core maps."""
    import ml_dtypes

    f32 = np.float32
    bf16 = ml_dtypes.bfloat16

    def t2(w):  # [out,in] -> [in,out]
        return np.ascontiguousarray(np.asarray(w, f32).T)

    def pairblk(w):
        """[8,64,64] per-head W -> [4,128,128] block-diag pair lhsT:
        blkdiag(W[2g].T, W[2g+1].T)."""
        wt = np.transpose(np.asarray(w, f32), (0, 2, 1))
        out = np.zeros((H // 2, P, P), f32)
        for g in range(H // 2):
            out[g, :HD, :HD] = wt[2 * g]
            out[g, HD:, HD:] = wt[2 * g + 1]
        return out.astype(bf16)

    a_val = float(1.0 / (1.0 + np.exp(-np.float32(inputs["alpha"]))))
    mask = np.asarray(inputs["mask"], f32)
    with_mask = bool(np.any(mask))

    wp = np.asarray(inputs["W_proj_in"], f32)
    wsi = np.asarray(inputs["W_split_inter"], f32)
    woi = np.asarray(inputs["W_out_inter"], f32)
    wpo = np.asarray(inputs["W_proj_out"], f32)
    win = wsi @ wp  # a2T = Win @ x^T
    wout = wpo @ woi  # out_inter = Wout @ concat^T (+x)

    common = {
        "WinT": t2(win).astype(bf16),
        "WoutT": t2(wout).astype(bf16),
        "WsaT": t2(inputs["W_split_intra"]).astype(bf16),
        "WoaT": np.ascontiguousarray(
            (np.asarray(inputs["W_out_intra"], f32) * f32(1.0 - a_val)).T
        ).astype(bf16),
        "ident": np.eye(P, dtype=f32).astype(bf16),
        "WqPi": pairblk(inputs["Wq_inter"]),
        "WkPi": pairblk(inputs["Wk_inter"]),
        "WvPi": pairblk(inputs["Wv_inter"]),
        "WqPa": pairblk(inputs["Wq_intra"]),
        "WkPa": pairblk(inputs["Wk_intra"]),
        "WvPa": pairblk(inputs["Wv_intra"]),
    }
    if with_mask:
        common["maskT"] = np.ascontiguousarray(mask.T)

    x = np.asarray(inputs["x"], f32)
    xb = x.astype(bf16)
    xtb = np.ascontiguousarray(np.swapaxes(x, 1, 2)).astype(bf16)
    in_maps = []
    for c in range(NCORES):
        m = dict(common)
        m["xb"] = np.ascontiguousarray(xb[c * BPC : (c + 1) * BPC])
        m["xtb"] = np.ascontiguousarray(xtb[c * BPC : (c + 1) * BPC])
        in_maps.append(m)
    return a_val, with_mask, in_maps


def _run(inputs, trace=False):
    a_val, with_mask, in_maps = _prep_inputs(inputs)
    nc = build_bass(a_val, with_mask)
    res = run_bass_kernel_spmd(
        nc,
        in_maps,
        core_ids=list(range(NCORES)),
        trace=trace,
    )
    out = np.concatenate([res.results[c]["y"] for c in range(NCORES)], axis=0)
    return out.astype(np.float32), res


def kernel(**inputs):
    out, _ = _run(inputs, trace=False)
    return out
